# revision 9
# baseline (speedup 1.0000x reference)
"""DenseCRF mean-field (10 iter) Trainium2 kernel, 8-core data parallel over B.

Self-contained: hardcodes shapes from the problem spec:
  unary [8,21,512,512] f32, image [8,3,512,512] f32, compatibility=I[21],
  spatial_weight=3.0, bilateral_weight=5.0 -> out [8,21,512,512] f32.

Device algorithm per core (one batch image), H on partitions:
  state Q in DRAM ping-pong buffers (padded [640,21,516] bf16, zero guards).
  Per iteration, 5 row-tiles (124 fresh rows each, 2-row vertical halo via
  padded DRAM reads). Per tile: Qe = Q*edge; per class: 5x5 box sum of Q and
  3x3 box sum of Qe via banded matmuls with horizontally shifted rhs windows
  accumulating in PSUM; bilateral normalizer fold: t = S3(Qe)*inv2 with
  inv2 = (25*bw/sw)/(S3(edge)+9e-6); inject t into the S5 PSUM via identity
  matmul; h = exp(-(sw/25)*PSUM) on ScalarE; E = exp(unary)*h; Z = class-sum;
  Q' = E/Z.  (compat = identity folded away; exp(u - m) = exp(u)*exp(-m).)
"""
import numpy as np
import ml_dtypes
from contextlib import ExitStack

import concourse.bass as bass
import concourse.tile as tile
from concourse import bacc, mybir
from concourse.bass_utils import run_bass_kernel_spmd

BF = ml_dtypes.bfloat16

B, C, H, W = 8, 21, 512, 512
WP = W + 4            # padded width (2 guard cols each side)
HP = 640              # padded rows (2 top guards + 512 + slack)
FRESH = 124           # fresh rows per tile
N_TILES = 5           # ceil(512/124)
N_ITER = 10


def _fr(t):
    return min(FRESH, H - FRESH * t)


def build_nc(sw: float, bw: float, n_iter: int = N_ITER, debug: bool = False):
    swp = sw / 25.0
    nc = bacc.Bacc("TRN2", target_bir_lowering=False, debug=debug, num_devices=8)
    bf = mybir.dt.bfloat16
    f32 = mybir.dt.float32

    eu_d = nc.declare_dram_parameter("eu", [HP, C, W], bf, isOutput=False)
    q0p_d = nc.declare_dram_parameter("q0p", [HP, C, WP], bf, isOutput=False)
    ep_d = nc.declare_dram_parameter("ep", [HP, WP], bf, isOutput=False)
    inv2p_d = nc.declare_dram_parameter("inv2p", [HP, W], f32, isOutput=False)
    bands_d = nc.declare_dram_parameter("bands", [3, 128, 128], bf, isOutput=False)
    qout_d = nc.declare_dram_parameter("qout", [C, H, W], f32, isOutput=True)
    qb_d = nc.dram_tensor("qb", [HP, C, WP], bf)
    qa_d = nc.dram_tensor("qa", [HP, C, WP], bf)

    with tile.TileContext(nc) as tc:
        with ExitStack() as ctx:
            res = ctx.enter_context(tc.tile_pool(name="res", bufs=1))
            qpool = ctx.enter_context(tc.tile_pool(name="qpool", bufs=2))
            eupool = ctx.enter_context(tc.tile_pool(name="eupool", bufs=2))
            big = ctx.enter_context(tc.tile_pool(name="big", bufs=1))
            small = ctx.enter_context(tc.tile_pool(name="small", bufs=2))
            tpool = ctx.enter_context(tc.tile_pool(name="tpool", bufs=3))
            psum = ctx.enter_context(tc.tile_pool(name="psum", bufs=4, space="PSUM"))

            # ---- resident constants
            band5 = res.tile([128, 128], bf, tag="band5")
            band3 = res.tile([128, 128], bf, tag="band3")
            ident = res.tile([128, 128], bf, tag="ident")
            nc.gpsimd.dma_start(out=band5, in_=bands_d.ap()[0])
            nc.gpsimd.dma_start(out=band3, in_=bands_d.ap()[1])
            nc.gpsimd.dma_start(out=ident, in_=bands_d.ap()[2])
            e_res = []
            i2_res = []
            for t in range(N_TILES):
                r0 = FRESH * t
                et = res.tile([128, WP], bf, tag=f"e{t}")
                nc.gpsimd.dma_start(out=et, in_=ep_d.ap()[r0:r0 + 128, :])
                it_ = res.tile([128, W], f32, tag=f"i2{t}")
                nc.gpsimd.dma_start(out=it_, in_=inv2p_d.ap()[r0:r0 + 128, :])
                e_res.append(et)
                i2_res.append(it_)

            # ---- prepass: qa <- q0p (padded, host-zeroed guards); qb guards <- 0
            zt = big.tile([128, C, WP], bf, tag="qe")  # reuse qe slot
            nc.vector.memset(zt, 0.0)
            for s in range(N_TILES):
                nc.gpsimd.dma_start(out=qb_d.ap()[128 * s:128 * (s + 1)], in_=zt)
            for s in range(N_TILES):
                nc.gpsimd.dma_start(out=qa_d.ap()[128 * s:128 * (s + 1)],
                                    in_=q0p_d.ap()[128 * s:128 * (s + 1)])

            def one_tile(t, qsrc, qdst, final):
                fr = _fr(t)
                r0 = FRESH * t
                qt = qpool.tile([128, C, WP], bf, tag="qt")
                nc.gpsimd.dma_start(out=qt, in_=qsrc.ap()[r0:r0 + 128])
                eut = eupool.tile([128, C, W], bf, tag="eut")
                nc.gpsimd.dma_start(out=eut, in_=eu_d.ap()[r0:r0 + 128])

                et, it_ = e_res[t], i2_res[t]
                qe = big.tile([128, C, WP], bf, tag="qe")
                e_b = bass.AP(tensor=et.tensor, offset=et.offset,
                              ap=[et.ap[0], [0, C], [1, WP]])
                nc.vector.tensor_mul(qe, qt, e_b)

                hfull = big.tile([128, C, W], bf, tag="hfull")
                for c in range(C):
                    p5 = psum.tile([128, W], mybir.dt.float32, tag="p5")
                    p3 = psum.tile([128, W], mybir.dt.float32, tag="p3")
                    for i, dx in enumerate((-2, -1, 0, 1, 2)):
                        nc.tensor.matmul(p5, band5, qt[:, c, 2 + dx:2 + dx + W],
                                         start=(i == 0), stop=False)
                    for i, dx in enumerate((-1, 0, 1)):
                        nc.tensor.matmul(p3, band3, qe[:, c, 2 + dx:2 + dx + W],
                                         start=(i == 0), stop=(i == 2))
                    tb = tpool.tile([128, W], bf, tag="tb")
                    nc.vector.tensor_mul(tb, p3, it_)
                    nc.tensor.matmul(p5, ident, tb, start=False, stop=True)
                    nc.scalar.activation(out=hfull[:, c, :], in_=p5,
                                         func=mybir.ActivationFunctionType.Exp,
                                         scale=-swp)

                ee = big.tile([128, C, W], bf, tag="ee")
                nc.vector.tensor_mul(ee, eut, hfull)
                zz = small.tile([128, W], mybir.dt.float32, tag="zz")
                e_reord = bass.AP(tensor=ee.tensor, offset=ee.offset,
                                  ap=[ee.ap[0], [1, W], [W, C]])
                nc.vector.tensor_reduce(zz, e_reord, axis=mybir.AxisListType.X,
                                        op=mybir.AluOpType.add)
                rr = small.tile([128, W], mybir.dt.float32, tag="rr")
                nc.vector.reciprocal(rr, zz)
                if not final:
                    rb = small.tile([128, W], bf, tag="rb")
                    nc.vector.tensor_copy(rb, rr)
                    qp = big.tile([128, C, W], bf, tag="hfull")  # reuse hfull slot
                    rb_b = bass.AP(tensor=rb.tensor, offset=rb.offset,
                                   ap=[rb.ap[0], [0, C], [1, W]])
                    nc.vector.tensor_mul(qp, ee, rb_b)
                    nc.gpsimd.dma_start(
                        out=qdst.ap()[r0 + 2:r0 + 2 + fr, :, 2:2 + W],
                        in_=qp[2:2 + fr])
                else:
                    for c in range(C):
                        fo = tpool.tile([128, W], mybir.dt.float32, tag="fo")
                        nc.vector.tensor_mul(fo, ee[:, c, :], rr)
                        nc.gpsimd.dma_start(out=qout_d.ap()[c, r0:r0 + fr, :],
                                            in_=fo[2:2 + fr])

            def one_iter(qsrc, qdst, final=False):
                for t in range(N_TILES):
                    one_tile(t, qsrc, qdst, final)

            pairs = (n_iter - 2) // 2
            if pairs > 0:
                with tc.For_i(0, pairs, 1):
                    one_iter(qa_d, qb_d)
                    one_iter(qb_d, qa_d)
            one_iter(qa_d, qb_d)
            one_iter(qb_d, None, final=True)

    nc.compile()
    return nc


def _edge_np(img):
    """image [3,H,W] f32 -> edge [H,W] f32, matching the jax reference."""
    gray = (0.299 * img[0] + 0.587 * img[1] + 0.114 * img[2]).astype(np.float32)
    gp = np.zeros((H + 2, W + 2), dtype=np.float32)
    gp[1:H + 1, 1:W + 1] = gray
    sx = np.array([[-1, 0, 1], [-2, 0, 2], [-1, 0, 1]], dtype=np.float32)
    sy = np.array([[-1, -2, -1], [0, 0, 0], [1, 2, 1]], dtype=np.float32)
    gx = np.zeros((H, W), dtype=np.float32)
    gy = np.zeros((H, W), dtype=np.float32)
    for i in range(3):
        for j in range(3):
            if sx[i, j]:
                gx += sx[i, j] * gp[i:i + H, j:j + W]
            if sy[i, j]:
                gy += sy[i, j] * gp[i:i + H, j:j + W]
    mag = np.sqrt(gx * gx + gy * gy + np.float32(1e-6))
    return np.exp(-mag).astype(np.float32)


def _box3(x):
    xp = np.zeros((H + 2, W + 2), dtype=np.float64)
    xp[1:H + 1, 1:W + 1] = x
    out = np.zeros((H, W), dtype=np.float64)
    for dy in range(3):
        for dx in range(3):
            out += xp[dy:dy + H, dx:dx + W]
    return out.astype(np.float32)


def _numpy_fallback(unary, image, compatibility, sw, bw):
    from scipy.ndimage import uniform_filter  # pragma: no cover
    raise NotImplementedError


_NC_CACHE = {}


def kernel(unary, image, compatibility, spatial_weight, bilateral_weight):
    unary = np.asarray(unary, dtype=np.float32)
    image = np.asarray(image, dtype=np.float32)
    compatibility = np.asarray(compatibility, dtype=np.float32)
    sw = max(float(spatial_weight), 0.0)
    bw = max(float(bilateral_weight), 0.0)
    assert np.allclose(compatibility, np.eye(C, dtype=np.float32)), \
        "kernel specialized to identity compatibility"
    assert sw > 0.0

    key = (sw, bw)
    if key not in _NC_CACHE:
        _NC_CACHE[key] = build_nc(sw, bw)
    nc = _NC_CACHE[key]

    # band matrices
    bands = np.zeros((3, 128, 128), dtype=BF)
    for k in range(128):
        for m in range(128):
            if abs(k - m) <= 2:
                bands[0, k, m] = 1.0
            if abs(k - m) <= 1:
                bands[1, k, m] = 1.0
        bands[2, k, k] = 1.0

    in_maps = []
    for b in range(B):
        u = unary[b]                                  # [C,H,W]
        e = _edge_np(image[b])                        # [H,W]
        s3e = _box3(e)
        inv2 = (25.0 * bw / sw) / (s3e + np.float32(9e-6))
        inv2p = np.zeros((HP, W), dtype=np.float32)
        inv2p[2:2 + H] = inv2
        ep = np.zeros((HP, WP), dtype=BF)
        ep[2:2 + H, 2:2 + W] = e.astype(BF)
        eu = np.exp(u).astype(BF)                     # [C,H,W]
        eu_hcw = np.full((HP, C, W), 1e-30, dtype=BF)       # padded [HP,C,W]
        eu_hcw[2:2 + H] = eu.transpose(1, 0, 2)
        m = u.max(axis=0, keepdims=True)
        q0 = np.exp(u - m)
        q0 = (q0 / q0.sum(axis=0, keepdims=True)).astype(BF)   # [C,H,W]
        q0p = np.zeros((HP, C, WP), dtype=BF)
        q0p[2:2 + H, :, 2:2 + W] = q0.transpose(1, 0, 2)
        in_maps.append({
            "eu": eu_hcw, "q0p": q0p, "ep": ep, "inv2p": inv2p,
            "bands": bands,
        })

    global LAST_RESULT
    res = run_bass_kernel_spmd(nc, in_maps, core_ids=list(range(B)),
                               trace=TRACE, trace_cores=[0] if TRACE else None)
    LAST_RESULT = res
    out = np.stack([res.results[b]["qout"] for b in range(B)], axis=0)
    return out.astype(np.float32)


TRACE = False
LAST_RESULT = None


# revision 10
# speedup vs baseline: 1.1461x; 1.1461x over previous
"""DenseCRF mean-field (10 iter) Trainium2 kernel, 8-core data parallel over B.

Self-contained: hardcodes shapes from the problem spec:
  unary [8,21,512,512] f32, image [8,3,512,512] f32, compatibility=I[21],
  spatial_weight=3.0, bilateral_weight=5.0 -> out [8,21,512,512] f32.

Device algorithm per core (one batch image), H on partitions:
  state Q in DRAM ping-pong buffers (padded [640,21,516] bf16, zero guards).
  Per iteration, 5 row-tiles (124 fresh rows each, 2-row vertical halo via
  padded DRAM reads). Per tile: Qe = Q*edge; per class: 5x5 box sum of Q and
  3x3 box sum of Qe via banded matmuls with horizontally shifted rhs windows
  accumulating in PSUM; bilateral normalizer fold: t = S3(Qe)*inv2 with
  inv2 = (25*bw/sw)/(S3(edge)+9e-6); inject t into the S5 PSUM via identity
  matmul; h = exp(-(sw/25)*PSUM) on ScalarE; E = exp(unary)*h; Z = class-sum;
  Q' = E/Z.  (compat = identity folded away; exp(u - m) = exp(u)*exp(-m).)
"""
import numpy as np
import ml_dtypes
from contextlib import ExitStack

import concourse.bass as bass
import concourse.tile as tile
from concourse import bacc, mybir
from concourse.bass_utils import run_bass_kernel_spmd

BF = ml_dtypes.bfloat16

B, C, H, W = 8, 21, 512, 512
WP = W + 4            # padded width (2 guard cols each side)
HP = 640              # padded rows (2 top guards + 512 + slack)
FRESH = 124           # fresh rows per tile
N_TILES = 5           # ceil(512/124)
N_ITER = 10


def _fr(t):
    return min(FRESH, H - FRESH * t)


def build_nc(sw: float, bw: float, n_iter: int = N_ITER, debug: bool = False):
    swp = sw / 25.0
    nc = bacc.Bacc("TRN2", target_bir_lowering=False, debug=debug, num_devices=8)
    bf = mybir.dt.bfloat16
    f32 = mybir.dt.float32

    eu_d = nc.declare_dram_parameter("eu", [HP, C, W], bf, isOutput=False)
    q0p_d = nc.declare_dram_parameter("q0p", [HP, C, WP], bf, isOutput=False)
    ep_d = nc.declare_dram_parameter("ep", [HP, WP], bf, isOutput=False)
    inv2p_d = nc.declare_dram_parameter("inv2p", [HP, W], f32, isOutput=False)
    bands_d = nc.declare_dram_parameter("bands", [3, 128, 128], bf, isOutput=False)
    qout_d = nc.declare_dram_parameter("qout", [C, H, W], f32, isOutput=True)
    qb_d = nc.dram_tensor("qb", [HP, C, WP], bf)
    qa_d = nc.dram_tensor("qa", [HP, C, WP], bf)

    with tile.TileContext(nc) as tc:
        with ExitStack() as ctx:
            res = ctx.enter_context(tc.tile_pool(name="res", bufs=1))
            qpool = ctx.enter_context(tc.tile_pool(name="qpool", bufs=2))
            eupool = ctx.enter_context(tc.tile_pool(name="eupool", bufs=2))
            big = ctx.enter_context(tc.tile_pool(name="big", bufs=1))
            small = ctx.enter_context(tc.tile_pool(name="small", bufs=2))
            tpool = ctx.enter_context(tc.tile_pool(name="tpool", bufs=3))
            psum = ctx.enter_context(tc.tile_pool(name="psum", bufs=4, space="PSUM"))

            # ---- resident constants
            band5 = res.tile([128, 128], bf, tag="band5")
            band3 = res.tile([128, 128], bf, tag="band3")
            ident = res.tile([128, 128], bf, tag="ident")
            nc.gpsimd.dma_start(out=band5, in_=bands_d.ap()[0])
            nc.gpsimd.dma_start(out=band3, in_=bands_d.ap()[1])
            nc.gpsimd.dma_start(out=ident, in_=bands_d.ap()[2])
            e_res = []
            i2_res = []
            for t in range(N_TILES):
                r0 = FRESH * t
                et = res.tile([128, WP], bf, tag=f"e{t}")
                nc.gpsimd.dma_start(out=et, in_=ep_d.ap()[r0:r0 + 128, :])
                it_ = res.tile([128, W], f32, tag=f"i2{t}")
                nc.gpsimd.dma_start(out=it_, in_=inv2p_d.ap()[r0:r0 + 128, :])
                e_res.append(et)
                i2_res.append(it_)

            # ---- prepass: qa <- q0p (padded, host-zeroed guards); qb guards <- 0
            zt = big.tile([128, C, WP], bf, tag="qe")  # reuse qe slot
            nc.vector.memset(zt, 0.0)
            for s in range(N_TILES):
                nc.gpsimd.dma_start(out=qb_d.ap()[128 * s:128 * (s + 1)], in_=zt)
            for s in range(N_TILES):
                nc.gpsimd.dma_start(out=qa_d.ap()[128 * s:128 * (s + 1)],
                                    in_=q0p_d.ap()[128 * s:128 * (s + 1)])

            def one_tile(t, qsrc, qdst, final):
                fr = _fr(t)
                r0 = FRESH * t
                qt = qpool.tile([128, C, WP], bf, tag="qt")
                nc.gpsimd.dma_start(out=qt, in_=qsrc.ap()[r0:r0 + 128])
                eut = eupool.tile([128, C, W], bf, tag="eut")
                nc.gpsimd.dma_start(out=eut, in_=eu_d.ap()[r0:r0 + 128])

                et, it_ = e_res[t], i2_res[t]
                qe = big.tile([128, C, WP], bf, tag="qe")
                e_b = bass.AP(tensor=et.tensor, offset=et.offset,
                              ap=[et.ap[0], [0, C], [1, WP]])
                nc.vector.tensor_mul(qe, qt, e_b)

                hfull = big.tile([128, C, W], bf, tag="hfull")
                for c in range(C):
                    p5 = psum.tile([128, W], mybir.dt.float32, tag="p5")
                    p3 = psum.tile([128, W], mybir.dt.float32, tag="p3")
                    for i, dx in enumerate((-2, -1, 0, 1, 2)):
                        nc.tensor.matmul(p5, band5, qt[:, c, 2 + dx:2 + dx + W],
                                         start=(i == 0), stop=False)
                    for i, dx in enumerate((-1, 0, 1)):
                        nc.tensor.matmul(p3, band3, qe[:, c, 2 + dx:2 + dx + W],
                                         start=(i == 0), stop=(i == 2))
                    tb = tpool.tile([128, W], bf, tag="tb")
                    nc.vector.tensor_mul(tb, p3, it_)
                    nc.tensor.matmul(p5, ident, tb, start=False, stop=True)
                    nc.scalar.activation(out=hfull[:, c, :], in_=p5,
                                         func=mybir.ActivationFunctionType.Exp,
                                         scale=-swp)

                ee = big.tile([128, C, W], bf, tag="ee")
                nc.vector.tensor_mul(ee, eut, hfull)
                zz = small.tile([128, W], mybir.dt.float32, tag="zz")
                e_reord = bass.AP(tensor=ee.tensor, offset=ee.offset,
                                  ap=[ee.ap[0], [1, W], [W, C]])
                nc.vector.tensor_reduce(zz, e_reord, axis=mybir.AxisListType.X,
                                        op=mybir.AluOpType.add)
                rr = small.tile([128, W], mybir.dt.float32, tag="rr")
                nc.vector.reciprocal(rr, zz)
                if not final:
                    rb = small.tile([128, W], bf, tag="rb")
                    nc.vector.tensor_copy(rb, rr)
                    qp = big.tile([128, C, W], bf, tag="hfull")  # reuse hfull slot
                    rb_b = bass.AP(tensor=rb.tensor, offset=rb.offset,
                                   ap=[rb.ap[0], [0, C], [1, W]])
                    nc.vector.tensor_mul(qp, ee, rb_b)
                    nc.gpsimd.dma_start(
                        out=qdst.ap()[r0 + 2:r0 + 2 + fr, :, 2:2 + W],
                        in_=qp[2:2 + fr])
                else:
                    for c in range(C):
                        fo = tpool.tile([128, W], mybir.dt.float32, tag="fo")
                        nc.vector.tensor_mul(fo, ee[:, c, :], rr)
                        nc.gpsimd.dma_start(out=qout_d.ap()[c, r0:r0 + fr, :],
                                            in_=fo[2:2 + fr])

            def one_iter(qsrc, qdst, final=False):
                for t in range(N_TILES):
                    one_tile(t, qsrc, qdst, final)

            pairs = (n_iter - 2) // 2
            if pairs > 0:
                with tc.For_i(0, pairs, 1):
                    one_iter(qa_d, qb_d)
                    one_iter(qb_d, qa_d)
            one_iter(qa_d, qb_d)
            one_iter(qb_d, None, final=True)

    nc.compile()
    return nc


def _edge_np(img):
    """image [3,H,W] f32 -> edge [H,W] f32, matching the jax reference."""
    gray = (0.299 * img[0] + 0.587 * img[1] + 0.114 * img[2]).astype(np.float32)
    gp = np.zeros((H + 2, W + 2), dtype=np.float32)
    gp[1:H + 1, 1:W + 1] = gray
    sx = np.array([[-1, 0, 1], [-2, 0, 2], [-1, 0, 1]], dtype=np.float32)
    sy = np.array([[-1, -2, -1], [0, 0, 0], [1, 2, 1]], dtype=np.float32)
    gx = np.zeros((H, W), dtype=np.float32)
    gy = np.zeros((H, W), dtype=np.float32)
    for i in range(3):
        for j in range(3):
            if sx[i, j]:
                gx += sx[i, j] * gp[i:i + H, j:j + W]
            if sy[i, j]:
                gy += sy[i, j] * gp[i:i + H, j:j + W]
    mag = np.sqrt(gx * gx + gy * gy + np.float32(1e-6))
    return np.exp(-mag).astype(np.float32)


def _box3(x):
    xp = np.zeros((H + 2, W + 2), dtype=np.float64)
    xp[1:H + 1, 1:W + 1] = x
    out = np.zeros((H, W), dtype=np.float64)
    for dy in range(3):
        for dx in range(3):
            out += xp[dy:dy + H, dx:dx + W]
    return out.astype(np.float32)


_NC_CACHE = {}


def kernel(unary, image, compatibility, spatial_weight, bilateral_weight):
    unary = np.asarray(unary, dtype=np.float32)
    image = np.asarray(image, dtype=np.float32)
    compatibility = np.asarray(compatibility, dtype=np.float32)
    sw = max(float(spatial_weight), 0.0)
    bw = max(float(bilateral_weight), 0.0)
    assert np.allclose(compatibility, np.eye(C, dtype=np.float32)), \
        "kernel specialized to identity compatibility"
    assert sw > 0.0

    key = (sw, bw)
    if key not in _NC_CACHE:
        _NC_CACHE[key] = build_nc(sw, bw)
    nc = _NC_CACHE[key]

    # band matrices
    bands = np.zeros((3, 128, 128), dtype=BF)
    for k in range(128):
        for m in range(128):
            if abs(k - m) <= 2:
                bands[0, k, m] = 1.0
            if abs(k - m) <= 1:
                bands[1, k, m] = 1.0
        bands[2, k, k] = 1.0

    in_maps = []
    for b in range(B):
        u = unary[b]                                  # [C,H,W]
        e = _edge_np(image[b])                        # [H,W]
        s3e = _box3(e)
        inv2 = (25.0 * bw / sw) / (s3e + np.float32(9e-6))
        inv2p = np.zeros((HP, W), dtype=np.float32)
        inv2p[2:2 + H] = inv2
        ep = np.zeros((HP, WP), dtype=BF)
        ep[2:2 + H, 2:2 + W] = e.astype(BF)
        eu = np.exp(u).astype(BF)                     # [C,H,W]
        eu_hcw = np.full((HP, C, W), 1e-30, dtype=BF)       # padded [HP,C,W]
        eu_hcw[2:2 + H] = eu.transpose(1, 0, 2)
        m = u.max(axis=0, keepdims=True)
        q0 = np.exp(u - m)
        q0 = (q0 / q0.sum(axis=0, keepdims=True)).astype(BF)   # [C,H,W]
        q0p = np.zeros((HP, C, WP), dtype=BF)
        q0p[2:2 + H, :, 2:2 + W] = q0.transpose(1, 0, 2)
        in_maps.append({
            "eu": eu_hcw, "q0p": q0p, "ep": ep, "inv2p": inv2p,
            "bands": bands,
        })

    global LAST_RESULT
    res = run_bass_kernel_spmd(nc, in_maps, core_ids=list(range(B)),
                               trace=TRACE, trace_cores=[0] if TRACE else None)
    LAST_RESULT = res
    out = np.stack([res.results[b]["qout"] for b in range(B)], axis=0)
    return out.astype(np.float32)


TRACE = False
LAST_RESULT = None
